# revision 5
# baseline (speedup 1.0000x reference)
"""GNN message-passing kernel for 8 Trainium2 NeuronCores.

Strategy (edge/graph parallel, per the sharding hint):
 - Nodes are sharded 8 ways (12500/core, padded to 12544 = 98 tiles of 128).
 - Each core computes h1 = A1@w_gc1+b and a1 = (A1@Lin1+b)*n_param for its
   node slice, then the h1 slices are AllGathered.
 - The two SpMMs (segment_sum of w_e * h[src_e] over edge lists) are dst-
   sharded: each core owns the edges targeting its node slice.  Edges are
   host-sorted by dst tile and cut into 128-edge chunks; per chunk the core
   gathers h[src] rows with an indirect DMA and multiplies a DVE-built
   one-hot(dst-local)*w matrix against them on the TensorEngine, accumulating
   each dst tile in PSUM.  This computes the exact segment sum.
 - The downstream batched gather+MLP is data-parallel over the 4096-row
   batch (512 rows/core) after an AllGather of x2.  The (concat, weight2,
   lin1_w) chain is refactored into 50 per-position [128,128] matrices
   V_l = weight2 @ (L2[l] + L2[50]) so the whole head is one accumulated
   matmul chain per batch tile.
"""
import sys
sys.path.insert(0, "/opt/trn_rl_repo")
import numpy as np

N = 100000
E = 1600000
EMB = 256
D = 128
B = 4096
L = 50
NCLASS = 10
NCORES = 8
P = 128
S_PER = 12500           # real nodes per core
S_PAD = 12544           # padded nodes per core (98 tiles)
NT = S_PAD // P         # 98 dst/row tiles per core
NP_PAD = NCORES * S_PAD # padded global node count
BT = B // NCORES // P   # 4 batch tiles per core


def _remap(ids):
    """global node id -> padded global id"""
    return (ids // S_PER) * S_PAD + (ids % S_PER)


def _prep_edges(edge_src, edge_dst, edge_w):
    """Bucket edges by dst core, sort by dst tile, chunk to 128, pad chunk
    counts so every core has the same per-tile chunk count (SPMD program).
    Returns (nch_t [NT], per-core dicts of srcs/dl/ww [128, NCH])."""
    core = edge_dst // S_PER
    loc = edge_dst % S_PER
    tile = loc // P
    dl = (loc % P).astype(np.float32)
    counts = np.zeros((NCORES, NT), dtype=np.int64)
    per_core = []
    for c in range(NCORES):
        m = core == c
        t_c = tile[m]
        order = np.argsort(t_c, kind="stable")
        per_core.append((t_c[order],
                         _remap(edge_src[m][order]).astype(np.int32),
                         dl[m][order], edge_w[m][order].astype(np.float32)))
        counts[c] = np.bincount(t_c, minlength=NT)
    nch_t = np.maximum(1, (np.max(counts, axis=0) + P - 1) // P).astype(np.int64)
    NCH = int(nch_t.sum())
    base = np.zeros(NT, dtype=np.int64)
    base[1:] = np.cumsum(nch_t)[:-1]
    cores = []
    for c in range(NCORES):
        t_c, s_c, d_c, w_c = per_core[c]
        srcs = np.zeros((P, NCH), dtype=np.int32)
        dloc = np.zeros((P, NCH), dtype=np.float32)
        ww = np.zeros((P, NCH), dtype=np.float32)
        # edges of tile t occupy slots [0, counts[c,t]) within its chunk block
        tstart = np.zeros(NT, dtype=np.int64)
        tstart[1:] = np.cumsum(counts[c])[:-1]
        pos = np.arange(t_c.shape[0]) - tstart[t_c]      # position within tile
        col = base[t_c] + pos // P
        row = pos % P
        srcs[row, col] = s_c
        dloc[row, col] = d_c
        ww[row, col] = w_c
        cores.append((srcs, dloc, ww))
    return nch_t, base, NCH, cores


def _build(nch_t, base, NCH):
    import concourse.bass as bass
    import concourse.bacc as bacc
    import concourse.tile as tile
    import concourse.mybir as mybir
    from concourse.masks import make_identity
    f32 = mybir.dt.float32
    i32 = mybir.dt.int32

    nc = bacc.Bacc("TRN2", target_bir_lowering=False, debug=False,
                   num_devices=NCORES, enable_asserts=False)
    # ---- I/O ----
    a1t = nc.dram_tensor("a1t", [P, 2 * S_PAD], f32, kind="ExternalInput")
    wg1 = nc.dram_tensor("wg1", [P, 2 * D], f32, kind="ExternalInput")
    li1 = nc.dram_tensor("li1", [P, 2 * D], f32, kind="ExternalInput")
    wg2 = nc.dram_tensor("wg2", [P, D], f32, kind="ExternalInput")
    b1r = nc.dram_tensor("b1r", [1, D], f32, kind="ExternalInput")
    b2r = nc.dram_tensor("b2r", [1, D], f32, kind="ExternalInput")
    lbr = nc.dram_tensor("lbr", [1, D], f32, kind="ExternalInput")
    npv = nc.dram_tensor("npv", [P, NT], f32, kind="ExternalInput")
    n1m = nc.dram_tensor("n1m", [P, NT], f32, kind="ExternalInput")
    srcs = nc.dram_tensor("srcs", [P, NCH], i32, kind="ExternalInput")
    dlt = nc.dram_tensor("dlt", [P, NCH], f32, kind="ExternalInput")
    wwt = nc.dram_tensor("wwt", [P, NCH], f32, kind="ExternalInput")
    bix = nc.dram_tensor("bix", [P, BT * L], i32, kind="ExternalInput")
    vmt = nc.dram_tensor("vmt", [P, L * D], f32, kind="ExternalInput")
    cvc = nc.dram_tensor("cvc", [P, 1], f32, kind="ExternalInput")
    cls = nc.dram_tensor("cls", [P, NCLASS], f32, kind="ExternalInput")
    cbr = nc.dram_tensor("cbr", [1, NCLASS], f32, kind="ExternalInput")
    selT = nc.dram_tensor("selT", [P, BT * P], f32, kind="ExternalOutput")
    amax = nc.dram_tensor("amax", [P, BT], mybir.dt.uint32, kind="ExternalOutput")

    with tile.TileContext(nc) as tc:
        with tc.tile_pool(name="const", bufs=1) as cp, \
             tc.tile_pool(name="ld", bufs=3) as ld, \
             tc.tile_pool(name="gat", bufs=6) as gat, \
             tc.tile_pool(name="wrk", bufs=4) as wrk, \
             tc.tile_pool(name="psA", bufs=2, space="PSUM") as psA, \
             tc.tile_pool(name="psB", bufs=2, space="PSUM") as psB, \
             tc.tile_pool(name="psT", bufs=2, space="PSUM") as psT, \
             tc.tile_pool(name="dram", bufs=1, space="DRAM") as dram:

            # ---- constants / preloads ----
            ident = cp.tile([P, P], f32)
            make_identity(nc, ident[:])
            iota_i = cp.tile([P, P], i32)
            nc.gpsimd.iota(iota_i[:], pattern=[[1, P]], base=0, channel_multiplier=0)
            iota_f = cp.tile([P, P], f32)
            nc.vector.tensor_copy(iota_f[:], iota_i[:])

            wg1_sb = cp.tile([P, 2 * D], f32)
            nc.sync.dma_start(out=wg1_sb[:], in_=wg1.ap())
            li1_sb = cp.tile([P, 2 * D], f32)
            nc.sync.dma_start(out=li1_sb[:], in_=li1.ap())
            wg2_sb = cp.tile([P, D], f32)
            nc.sync.dma_start(out=wg2_sb[:], in_=wg2.ap())
            npv_sb = cp.tile([P, NT], f32)
            nc.sync.dma_start(out=npv_sb[:], in_=npv.ap())
            n1m_sb = cp.tile([P, NT], f32)
            nc.sync.dma_start(out=n1m_sb[:], in_=n1m.ap())
            src_sb = cp.tile([P, NCH], i32)
            nc.sync.dma_start(out=src_sb[:], in_=srcs.ap())
            dl_sb = cp.tile([P, NCH], f32)
            nc.sync.dma_start(out=dl_sb[:], in_=dlt.ap())
            ww_sb = cp.tile([P, NCH], f32)
            nc.sync.dma_start(out=ww_sb[:], in_=wwt.ap())
            bix_sb = cp.tile([P, BT * L], i32)
            nc.sync.dma_start(out=bix_sb[:], in_=bix.ap())
            vmt_sb = cp.tile([P, L * D], f32)
            nc.sync.dma_start(out=vmt_sb[:], in_=vmt.ap())
            cvc_sb = cp.tile([P, 1], f32)
            nc.sync.dma_start(out=cvc_sb[:], in_=cvc.ap())
            cls_sb = cp.tile([P, NCLASS], f32)
            nc.sync.dma_start(out=cls_sb[:], in_=cls.ap())

            ones1 = cp.tile([1, P], f32)
            nc.vector.memset(ones1[:], 1.0)

            def rep_bias(src_dram, width):
                row = cp.tile([1, width], f32, tag=f"rb_{src_dram.name}")
                nc.sync.dma_start(out=row[:], in_=src_dram.ap())
                ps = psT.tile([P, width], f32, space="PSUM", tag="aux")
                nc.tensor.matmul(ps[:], lhsT=ones1[:, :P], rhs=row[:],
                                 start=True, stop=True)
                out = cp.tile([P, width], f32, tag=f"rep_{src_dram.name}")
                nc.vector.tensor_copy(out[:], ps[:])
                return out

            b1_rep = rep_bias(b1r, D)
            b2_rep = rep_bias(b2r, D)
            lb_rep = rep_bias(lbr, D)
            cb_rep = rep_bias(cbr, NCLASS)

            # a1 for this core's slice, kept on-chip: [P, NT, D]
            a1_sb = cp.tile([P, NT * D], f32)

            # DRAM intermediates
            h1_c = dram.tile([S_PAD, D], f32)
            h2_c = dram.tile([S_PAD, D], f32)
            x2_c = dram.tile([S_PAD, D], f32)
            h1_full = dram.tile([NP_PAD, D], f32, addr_space="Shared")
            h2_full = dram.tile([NP_PAD, D], f32, addr_space="Shared")
            x2_full = dram.tile([NP_PAD, D], f32, addr_space="Shared")
            rg = [list(range(NCORES))]

            # ================= Phase A: h1 and a1 =================
            CH = 8  # row tiles per a1t load
            for g in range(0, NT, CH):
                w = min(CH, NT - g) * P
                at0 = ld.tile([P, CH * P], f32, tag="at0")
                at1 = ld.tile([P, CH * P], f32, tag="at1")
                nc.sync.dma_start(out=at0[:, :w], in_=a1t.ap()[:, g * P:g * P + w])
                nc.sync.dma_start(out=at1[:, :w], in_=a1t.ap()[:, S_PAD + g * P:S_PAD + g * P + w])
                for j in range(min(CH, NT - g)):
                    rt = g + j
                    lhs0 = at0[:, j * P:(j + 1) * P]
                    lhs1 = at1[:, j * P:(j + 1) * P]
                    ph = psA.tile([P, D], f32, space="PSUM", tag="ph")
                    nc.tensor.matmul(ph[:], lhsT=lhs0, rhs=wg1_sb[:, :D],
                                     start=True, stop=False)
                    nc.tensor.matmul(ph[:], lhsT=lhs1, rhs=wg1_sb[:, D:],
                                     start=False, stop=True)
                    pa = psA.tile([P, D], f32, space="PSUM", tag="pa")
                    nc.tensor.matmul(pa[:], lhsT=lhs0, rhs=li1_sb[:, :D],
                                     start=True, stop=False)
                    nc.tensor.matmul(pa[:], lhsT=lhs1, rhs=li1_sb[:, D:],
                                     start=False, stop=True)
                    h1_t = wrk.tile([P, D], f32, tag="h1t")
                    nc.vector.tensor_add(h1_t[:], ph[:], b1_rep[:])
                    nc.scalar.dma_start(out=h1_c[rt * P:(rt + 1) * P, :], in_=h1_t[:])
                    a0_t = wrk.tile([P, D], f32, tag="a0t")
                    nc.vector.tensor_add(a0_t[:], pa[:], lb_rep[:])
                    nc.vector.tensor_scalar(
                        out=a1_sb[:, rt * D:(rt + 1) * D], in0=a0_t[:],
                        scalar1=npv_sb[:, rt:rt + 1], scalar2=None,
                        op0=mybir.AluOpType.mult)
            nc.gpsimd.collective_compute(
                "AllGather", mybir.AluOpType.bypass, replica_groups=rg,
                ins=[h1_c[:]], outs=[h1_full[:]])

            # ================= SpMM helper =================
            def spmm_tile(t, htab, out_cb):
                """segment-sum tile t from htab into psum, then out_cb(psum)."""
                px = psB.tile([P, D], f32, space="PSUM", tag="px")
                nch = int(nch_t[t])
                for j in range(nch):
                    col = int(base[t]) + j
                    hg = gat.tile([P, D], f32, tag="hg")
                    nc.gpsimd.indirect_dma_start(
                        out=hg[:], out_offset=None, in_=htab[:],
                        in_offset=bass.IndirectOffsetOnAxis(
                            ap=src_sb[:, col:col + 1], axis=0))
                    oh = gat.tile([P, P], f32, tag="oh")
                    nc.vector.tensor_scalar(
                        out=oh[:], in0=iota_f[:],
                        scalar1=dl_sb[:, col:col + 1],
                        scalar2=ww_sb[:, col:col + 1],
                        op0=mybir.AluOpType.is_equal,
                        op1=mybir.AluOpType.mult)
                    nc.tensor.matmul(px[:], lhsT=oh[:], rhs=hg[:],
                                     start=(j == 0), stop=(j == nch - 1))
                out_cb(px)

            # ================= Phase B: x1 -> h2 =================
            for t in range(NT):
                def fin_b(px, t=t):
                    x1_t = wrk.tile([P, D], f32, tag="x1t")
                    nc.vector.tensor_scalar(
                        out=x1_t[:], in0=px[:],
                        scalar1=n1m_sb[:, t:t + 1], scalar2=None,
                        op0=mybir.AluOpType.mult)
                    nc.vector.tensor_add(x1_t[:], x1_t[:],
                                         a1_sb[:, t * D:(t + 1) * D])
                    ptr = psT.tile([P, D], f32, space="PSUM", tag="aux")
                    nc.tensor.transpose(ptr[:], x1_t[:], ident[:])
                    x1T = wrk.tile([P, D], f32, tag="x1T")
                    nc.vector.tensor_copy(x1T[:], ptr[:])
                    ph2 = psT.tile([P, D], f32, space="PSUM", tag="aux")
                    nc.tensor.matmul(ph2[:], lhsT=x1T[:], rhs=wg2_sb[:],
                                     start=True, stop=True)
                    h2_t = wrk.tile([P, D], f32, tag="h2t")
                    nc.vector.tensor_add(h2_t[:], ph2[:], b2_rep[:])
                    nc.scalar.dma_start(out=h2_c[t * P:(t + 1) * P, :], in_=h2_t[:])
                spmm_tile(t, h1_full, fin_b)
            nc.gpsimd.collective_compute(
                "AllGather", mybir.AluOpType.bypass, replica_groups=rg,
                ins=[h2_c[:]], outs=[h2_full[:]])

            # ================= Phase C: x2 =================
            for t in range(NT):
                def fin_c(px, t=t):
                    x2_t = wrk.tile([P, D], f32, tag="x2t")
                    nc.vector.tensor_copy(x2_t[:], px[:])
                    nc.scalar.dma_start(out=x2_c[t * P:(t + 1) * P, :], in_=x2_t[:])
                spmm_tile(t, h2_full, fin_c)
            nc.gpsimd.collective_compute(
                "AllGather", mybir.AluOpType.bypass, replica_groups=rg,
                ins=[x2_c[:]], outs=[x2_full[:]])

            # ================= Phase D: batch head =================
            for bt in range(BT):
                psel = psB.tile([P, D], f32, space="PSUM", tag="px")
                for l in range(L):
                    col = bt * L + l
                    gx = gat.tile([P, D], f32, tag="gx")
                    nc.gpsimd.indirect_dma_start(
                        out=gx[:], out_offset=None, in_=x2_full[:],
                        in_offset=bass.IndirectOffsetOnAxis(
                            ap=bix_sb[:, col:col + 1], axis=0))
                    ptr = psT.tile([P, D], f32, space="PSUM", tag="aux")
                    nc.tensor.transpose(ptr[:], gx[:], ident[:])
                    gT = wrk.tile([P, D], f32, tag="gT")
                    nc.vector.tensor_copy(gT[:], ptr[:])
                    nc.tensor.matmul(psel[:], lhsT=vmt_sb[:, l * D:(l + 1) * D],
                                     rhs=gT[:], start=(l == 0), stop=(l == L - 1))
                sT = wrk.tile([P, D], f32, tag="sT")
                nc.vector.tensor_scalar(
                    out=sT[:], in0=psel[:], scalar1=cvc_sb[:, :1], scalar2=None,
                    op0=mybir.AluOpType.add)
                nc.scalar.dma_start(out=selT.ap()[:, bt * P:(bt + 1) * P], in_=sT[:])
                plg = psT.tile([P, NCLASS], f32, space="PSUM", tag="aux")
                nc.tensor.matmul(plg[:], lhsT=sT[:], rhs=cls_sb[:],
                                 start=True, stop=True)
                lg = wrk.tile([P, NCLASS], f32, tag="lg")
                nc.vector.tensor_add(lg[:], plg[:], cb_rep[:])
                mx = wrk.tile([P, 8], f32, tag="mx")
                nc.vector.max(mx[:], lg[:])
                mi = wrk.tile([P, 8], mybir.dt.uint32, tag="mi")
                nc.vector.max_index(mi[:], mx[:], lg[:])
                nc.sync.dma_start(out=amax.ap()[:, bt:bt + 1], in_=mi[:, :1])
    nc.compile()
    return nc


def prepare(A1_tensor, edge_src, edge_dst, edge_w, batch_idx,
            w_gc1, b_gc1, w_gc2, b_gc2, n_param, Lin1, Lin1_bias,
            weight2, bias2, lin1_w, lin1_b, classifier, classifier_bias):
    A1_tensor = np.asarray(A1_tensor, dtype=np.float32)
    edge_src = np.asarray(edge_src, dtype=np.int32)
    edge_dst = np.asarray(edge_dst, dtype=np.int32)
    edge_w = np.asarray(edge_w, dtype=np.float32)
    batch_idx = np.asarray(batch_idx, dtype=np.int32)
    n_param = np.asarray(n_param, dtype=np.float32)

    nch_t, base, NCH, edge_cores = _prep_edges(edge_src, edge_dst, edge_w)
    nc = _build(nch_t, base, NCH)

    # V_l = weight2 @ (L2[l] + L2[50]); cvec = lin1_b + bias2 @ sum_l L2[l]
    L2 = np.asarray(lin1_w, np.float32).reshape(L + 1, 64, D)
    V = np.einsum("de,leh->ldh", np.asarray(weight2, np.float32),
                  L2[:L] + L2[L][None]).astype(np.float32)      # [L, 128, 128]
    vmt_np = V.transpose(1, 0, 2).reshape(D, L * D)             # [d, (l d')]
    cvec_np = (np.asarray(lin1_b, np.float32)
               + np.asarray(bias2, np.float32) @ L2.sum(axis=0)).reshape(D, 1)

    wg1_np = np.concatenate([np.asarray(w_gc1, np.float32)[:P],
                             np.asarray(w_gc1, np.float32)[P:]], axis=1)
    li1_np = np.concatenate([np.asarray(Lin1, np.float32)[:P],
                             np.asarray(Lin1, np.float32)[P:]], axis=1)

    bidx = batch_idx[:, 0, :]                                   # [B, L]
    bidx_pad = _remap(bidx.astype(np.int64)).astype(np.int32)

    in_maps = []
    for c in range(NCORES):
        sl = slice(c * S_PER, (c + 1) * S_PER)
        a1t_np = np.zeros((P, 2 * S_PAD), np.float32)
        a1t_np[:, :S_PER] = A1_tensor[sl, :P].T
        a1t_np[:, S_PAD:S_PAD + S_PER] = A1_tensor[sl, P:].T
        npv_np = np.zeros((P, NT), np.float32)
        n1m_np = np.zeros((P, NT), np.float32)
        flat = n_param[sl, 0]
        pad = np.zeros(S_PAD, np.float32)
        pad[:S_PER] = flat
        npv_np[:, :] = pad.reshape(NT, P).T
        n1m_np[:, :] = (1.0 - pad).reshape(NT, P).T
        srcs_np, dl_np, ww_np = edge_cores[c]
        bslice = bidx_pad[c * (B // NCORES):(c + 1) * (B // NCORES)]  # [512, L]
        bix_np = np.zeros((P, BT * L), np.int32)
        for bt in range(BT):
            bix_np[:, bt * L:(bt + 1) * L] = bslice[bt * P:(bt + 1) * P, :]
        in_maps.append({
            "a1t": a1t_np, "wg1": wg1_np, "li1": li1_np,
            "wg2": np.asarray(w_gc2, np.float32),
            "b1r": np.asarray(b_gc1, np.float32).reshape(1, D),
            "b2r": np.asarray(b_gc2, np.float32).reshape(1, D),
            "lbr": np.asarray(Lin1_bias, np.float32).reshape(1, D),
            "npv": npv_np, "n1m": n1m_np,
            "srcs": srcs_np, "dlt": dl_np, "wwt": ww_np,
            "bix": bix_np, "vmt": vmt_np, "cvc": cvec_np,
            "cls": np.asarray(classifier, np.float32),
            "cbr": np.asarray(classifier_bias, np.float32).reshape(1, NCLASS),
        })
    return nc, in_maps


def kernel(**inputs):
    from concourse import bass_utils
    nc, in_maps = prepare(**inputs)
    res = bass_utils.run_bass_kernel_spmd(nc, in_maps, core_ids=list(range(NCORES)))
    return _assemble(res.results)


def _assemble(results):
    sel = np.zeros((B, D), np.float32)
    am = np.zeros(B, np.int32)
    for c in range(NCORES):
        selT_c = results[c]["selT"]              # [128, BT*128]
        amax_c = results[c]["amax"]              # [128, BT] uint32
        for bt in range(BT):
            rows = slice(c * (B // NCORES) + bt * P,
                         c * (B // NCORES) + (bt + 1) * P)
            sel[rows] = selT_c[:, bt * P:(bt + 1) * P].T
            am[rows] = amax_c[:, bt].astype(np.int32)
    return am, sel
